# revision 12
# baseline (speedup 1.0000x reference)
"""Multi-head attention TRN2 kernel, sharded over 8 NeuronCores.

Sharding: (batch, head-group) — core c handles batch c//4 and heads
(c%4)*4 .. (c%4)*4+3. Each core computes its 4 heads' attention plus its
partial output projection; the host sums the 4 partials per batch and adds bo.

Device-side layout (per core):
  QT, KT: [hd=256, S] (bf16)   -- projections computed transposed
  V:      [S, 4 heads, 65]     -- 64 hd cols + ones col (softmax denominator)
  scores computed transposed [j, i] so the softmax sum and attn@V both
  contract over j (the partition dim) with no transposes of the big matrices.
  relative_pos_bias arrives pre-sliced per head in [h, j, i] layout (bf16)
  with the attention mask folded in as an additive -30000 (host-side prep).
  Matmuls run in bf16 (single-pass on the PE); all accumulation is fp32 in
  PSUM. Set PRECISE=1 to switch the matmul path to f32r (2-pass, ~2x slower,
  rel err ~3e-4 instead of ~4e-3).
"""
import os
import sys

if "/opt/trn_rl_repo" not in sys.path:
    sys.path.insert(0, "/opt/trn_rl_repo")

from contextlib import ExitStack

import ml_dtypes
import numpy as np

B, S, D, H = 2, 2048, 1024, 16
HD = D // H          # 64
NCORES = 8
HPC = 4              # heads per core
HDC = HPC * HD       # 256 head-dim cols per core
P = 128
ISLAB = 512          # i-columns per score slab
NJT = S // P         # 16 j tiles
NISLAB = S // ISLAB  # 4 i slabs
NSC = S // ISLAB     # 4 seq chunks in stage 0
MASK_NEG = np.float32(-30000.0)

PRECISE = os.environ.get("PRECISE", "0") == "1"

_CACHE = {}


def _build():
    import concourse.bass as bass
    import concourse.mybir as mybir
    import concourse.tile as tile
    from concourse.tile import add_dep_helper
    from concourse import bacc
    from concourse.masks import make_identity

    f32 = mybir.dt.float32
    mmdt = mybir.dt.float32r if PRECISE else mybir.dt.bfloat16
    biasdt = f32 if PRECISE else mybir.dt.bfloat16

    # All our ACT funcs (Exp, Ln, Copy, Identity) live together in the
    # 'natural_log_exp_and_others' table set; restricting the registry to it
    # makes insert_act_table_loads emit ONE load instead of thrashing
    # exp<->ln tables at every softmax-normalization point.
    import concourse.hw_specs as hw_specs
    if not getattr(hw_specs, "_mha_table_patch", False):
        _orig_gat = hw_specs.get_activation_tables

        def _one_table(arch, _orig=_orig_gat):
            t = _orig(arch)
            name = "natural_log_exp_and_others"
            if name not in t:
                return t
            keep = t[name]
            # preserve dict size/order (set index == act_func_set_id); just
            # make the shared funcs resolvable only via the ln+exp set
            return {
                k: (v if k == name else (v - keep))
                for k, v in t.items()
            }

        hw_specs.get_activation_tables = _one_table
        bacc.get_activation_tables = _one_table
        hw_specs._mha_table_patch = True

    nc = bacc.Bacc(None, target_bir_lowering=False)

    q_in = nc.declare_dram_parameter("q_in", [S, D], mmdt, isOutput=False)
    k_in = nc.declare_dram_parameter("k_in", [S, D], mmdt, isOutput=False)
    v_in = nc.declare_dram_parameter("v_in", [S, D], mmdt, isOutput=False)
    wq = nc.declare_dram_parameter("wq", [D, HDC], mmdt, isOutput=False)
    wk = nc.declare_dram_parameter("wk", [D, HDC], mmdt, isOutput=False)
    wv = nc.declare_dram_parameter("wv", [D, HDC], mmdt, isOutput=False)
    wo = nc.declare_dram_parameter("wo", [HDC, D], mmdt, isOutput=False)
    bq = nc.declare_dram_parameter("bq", [HDC], f32, isOutput=False)
    bk = nc.declare_dram_parameter("bk", [HDC], f32, isOutput=False)
    bv_rep = nc.declare_dram_parameter("bv_rep", [P, HDC], f32, isOutput=False)
    ones64 = nc.declare_dram_parameter("ones64", [1, 64], mmdt, isOutput=False)
    bias_c = nc.declare_dram_parameter(
        "bias_c", [HPC, NJT, NISLAB, P, ISLAB], biasdt, isOutput=False)
    out_p = nc.declare_dram_parameter("out_p", [S, D], f32, isOutput=True)

    EXP = mybir.ActivationFunctionType.Exp
    LN = mybir.ActivationFunctionType.Ln

    with tile.TileContext(nc) as tc, ExitStack() as big:
        consts = big.enter_context(tc.tile_pool(name="consts", bufs=1))
        persist = big.enter_context(tc.tile_pool(name="persist", bufs=1))

        ident = consts.tile([P, P], mmdt)
        make_identity(nc, ident)
        ones64_sb = consts.tile([1, 64], mmdt)
        nc.sync.dma_start(ones64_sb, ones64[:])
        bqv = consts.tile([P, 2], f32)
        nc.sync.dma_start(bqv, bq[:].rearrange("(o p) -> p o", p=P))
        bkv = consts.tile([P, 2], f32)
        nc.sync.dma_start(bkv, bk[:].rearrange("(o p) -> p o", p=P))
        bv_sb = consts.tile([P, HDC], f32)
        nc.sync.dma_start(bv_sb, bv_rep[:])
        ones_col = consts.tile([P, 1], f32)
        nc.vector.memset(ones_col, 1.0)

        wq_sb = consts.tile([P, 8, HDC], mmdt)
        nc.sync.dma_start(wq_sb, wq[:].rearrange("(dk p) m -> p dk m", p=P))
        wk_sb = consts.tile([P, 8, HDC], mmdt)
        nc.sync.dma_start(wk_sb, wk[:].rearrange("(dk p) m -> p dk m", p=P))
        wv_sb = consts.tile([P, 8, HDC], mmdt)
        nc.sync.dma_start(wv_sb, wv[:].rearrange("(dk p) m -> p dk m", p=P))
        wo_sb = consts.tile([P, 2, D], mmdt)
        nc.sync.dma_start(wo_sb, wo[:].rearrange("(kt p) n -> p kt n", p=P))

        qt_full = persist.tile([P, 2, S], mmdt)   # [hd%128, hd//128, seq]
        kt_full = persist.tile([P, 2, S], mmdt)
        v_full = persist.tile([P, NJT, HPC, HD + 1], mmdt)  # [seq%128, jt, h, hd|1]
        ctxT = persist.tile([P, 2, S], mmdt)      # [hd%128, hd//128, i]

        # ones column of V (softmax denominator trick)
        for jt in range(NJT):
            nc.vector.tensor_copy(
                v_full[:, jt, :, HD:HD + 1],
                ones_col[:, None, :].to_broadcast((P, HPC, 1)))

        # ---------------- Stage 0: transpose inputs + projections ----------
        s0 = big.enter_context(tc.tile_pool(name="s0", bufs=6))
        s0x = big.enter_context(tc.tile_pool(name="s0x", bufs=2))
        with ExitStack() as st0:
            tps = st0.enter_context(tc.tile_pool(name="tps", bufs=2, space="PSUM"))
            pps = st0.enter_context(tc.tile_pool(name="pps", bufs=2, space="PSUM"))
            vps = st0.enter_context(tc.tile_pool(name="vps", bufs=2, space="PSUM"))

            for x_dram, which in ((q_in, "q"), (k_in, "k"), (v_in, "v")):
                for sc in range(NSC):
                    xn = []
                    for st in range(4):
                        t = s0.tile([P, D], mmdt, tag="xn")
                        nc.sync.dma_start(t, x_dram[sc * ISLAB + st * P:
                                                    sc * ISLAB + (st + 1) * P, :])
                        xn.append(t)
                    xT = s0x.tile([P, 8, ISLAB], mmdt, tag="xT")
                    for dk in range(8):
                        tp4 = tps.tile([P, ISLAB], mmdt, tag="tp4")
                        for st in range(4):
                            nc.tensor.transpose(
                                tp4[:, st * P:(st + 1) * P],
                                xn[st][:, dk * P:(dk + 1) * P], ident)
                        if dk % 2 == 0:
                            nc.scalar.copy(xT[:, dk, :], tp4)
                        else:
                            nc.vector.tensor_copy(xT[:, dk, :], tp4)
                    if which == "v":
                        for st in range(4):
                            vp = vps.tile([P, HDC], f32, tag="vp")
                            for dk in range(8):
                                nc.tensor.matmul(
                                    vp, xT[:, dk, st * P:(st + 1) * P],
                                    wv_sb[:, dk, :],
                                    start=(dk == 0), stop=(dk == 7),
                                )
                            jt = sc * 4 + st
                            nc.vector.tensor_add(
                                v_full[:, jt, :, :HD],
                                vp.rearrange("p (h d) -> p h d", h=HPC),
                                bv_sb.rearrange("p (h d) -> p h d", h=HPC),
                            )
                    else:
                        dst = qt_full if which == "q" else kt_full
                        w_sb = wq_sb if which == "q" else wk_sb
                        bvec = bqv if which == "q" else bkv
                        for mt in range(2):
                            pp = pps.tile([P, ISLAB], f32, tag="pp")
                            for dk in range(8):
                                nc.tensor.matmul(
                                    pp, w_sb[:, dk, mt * P:(mt + 1) * P],
                                    xT[:, dk, :],
                                    start=(dk == 0), stop=(dk == 7),
                                )
                            nc.vector.tensor_scalar_add(
                                dst[:, mt, sc * ISLAB:(sc + 1) * ISLAB],
                                pp, bvec[:, mt:mt + 1],
                            )

        # ---------------- Stage 1: attention ------------------------------
        sbias = big.enter_context(tc.tile_pool(name="sbias", bufs=6))
        sein = big.enter_context(tc.tile_pool(name="sein", bufs=6))
        sexp = big.enter_context(tc.tile_pool(name="sexp", bufs=6))
        snrm = big.enter_context(tc.tile_pool(name="snrm", bufs=2))
        dnrm = big.enter_context(tc.tile_pool(name="dnrm", bufs=2, space="DRAM"))
        with ExitStack() as st1:
            sps = st1.enter_context(tc.tile_pool(name="sps", bufs=5, space="PSUM"))
            cps = st1.enter_context(tc.tile_pool(name="cps", bufs=3, space="PSUM"))

            LAG = 3

            def make_norm_steps(cp, hp, ho, isl):
                # Deferred softmax normalization for one finished slab:
                # 1/denom via exp(-ln(d)) on ACT, partition-broadcast via a
                # DRAM bounce, multiply on DVE. Emitted as discrete steps that
                # the caller interleaves into the NEXT slab's loop so these
                # never stall the in-order ACT/PE streams at slab boundaries.
                state = {}

                def s_ln():
                    state["lns"] = snrm.tile([1, ISLAB], f32, tag="lns", name="lns")
                    nc.scalar.activation(state["lns"], cp[HD:HD + 1, :], LN)

                def s_recip():
                    state["recip"] = snrm.tile([1, ISLAB], f32, tag="recip", name="recip")
                    nc.scalar.activation(state["recip"], state["lns"], EXP,
                                         scale=-1.0)

                def s_dma1():
                    state["dns"] = dnrm.tile([1, ISLAB], f32, tag="dns", name="dns")
                    nc.sync.dma_start(state["dns"], state["recip"])

                def s_dma2():
                    dns = state["dns"]
                    state["bsb"] = snrm.tile([64, ISLAB], f32, tag="bsb", name="bsb")
                    nc.sync.dma_start(state["bsb"], bass.AP(
                        tensor=dns.tensor, offset=dns.offset,
                        ap=[[0, 64]] + list(dns[0].ap)))

                def s_mul():
                    nc.vector.tensor_mul(
                        ctxT[hp:hp + 64, ho, isl * ISLAB:(isl + 1) * ISLAB],
                        cp[:HD, :], state["bsb"],
                    )

                return [s_ln, s_recip, s_dma1, s_dma2, s_mul]

            pending = []
            NORM_AT = {2: 0, 4: 1, 6: 2, 8: 3, 10: 4}
            for h in range(HPC):
                hp = (h % 2) * 64       # base partition of this head in qt/kt
                ho = h // 2             # outer index
                for isl in range(NISLAB):
                    qt_h = qt_full[hp:hp + 64, ho, isl * ISLAB:(isl + 1) * ISLAB]
                    cp = cps.tile([HD + 1, ISLAB], f32, tag="cp")
                    ets = [None] * NJT
                    sc_insts = [None] * NJT
                    for jt in range(NJT + LAG):
                        if jt in NORM_AT and pending:
                            pending[NORM_AT[jt]]()
                        if jt < NJT:
                            sp = sps.tile([P, ISLAB], f32, tag="sp")
                            smm = nc.tensor.matmul(
                                sp, kt_full[hp:hp + 64, ho, jt * P:(jt + 1) * P],
                                qt_h, start=True, stop=True,
                            )
                            sc_insts[jt] = smm
                            bt = sbias.tile([P, ISLAB], biasdt, tag="bt")
                            nc.sync.dma_start(bt, bias_c[h, jt, isl])
                            ein = sein.tile([P, ISLAB], f32, tag="ein")
                            nc.vector.tensor_add(ein, sp, bt)
                            et = sexp.tile([P, ISLAB], mmdt, tag="et")
                            nc.scalar.activation(et, ein, EXP)
                            ets[jt] = et
                        if jt >= LAG:
                            j2 = jt - LAG
                            cmm = nc.tensor.matmul(
                                cp, v_full[:, j2, h, :], ets[j2],
                                start=(j2 == 0), stop=(j2 == NJT - 1),
                            )
                            if jt < NJT:
                                # keep the software-pipeline skew in the PE
                                # stream: ctx(j2) goes AFTER scores(j2+LAG)
                                add_dep_helper(
                                    sc_insts[jt].ins, cmm.ins, sync=False,
                                    reason="preserve scores/ctx LAG skew")
                    pending = make_norm_steps(cp, hp, ho, isl)
            for step in pending:
                step()

        # ---------------- Stage 2: output projection ----------------------
        so = big.enter_context(tc.tile_pool(name="so", bufs=3))
        with ExitStack() as st2:
            ops = st2.enter_context(tc.tile_pool(name="ops", bufs=2, space="PSUM"))
            for it in range(S // P):
                for nt in range(2):
                    op = ops.tile([P, ISLAB], f32, tag="op")
                    for kt in range(2):
                        nc.tensor.matmul(
                            op, ctxT[:, kt, it * P:(it + 1) * P],
                            wo_sb[:, kt, nt * ISLAB:(nt + 1) * ISLAB],
                            start=(kt == 0), stop=(kt == 1),
                        )
                    ot = so.tile([P, ISLAB], f32, tag="ot")
                    nc.scalar.copy(ot, op)
                    nc.sync.dma_start(
                        out_p[it * P:(it + 1) * P,
                              nt * ISLAB:(nt + 1) * ISLAB], ot)

    nc.compile()
    return nc


def _get_nc():
    if "nc" not in _CACHE:
        _CACHE["nc"] = _build()
    return _CACHE["nc"]


def _prep_inputs(query, key, value, mask, relative_pos_bias,
                 Wq, bq, Wk, bk, Wv, bv, Wo, bo):
    f32 = np.float32
    mmdt = f32 if PRECISE else ml_dtypes.bfloat16
    biasdt = f32 if PRECISE else ml_dtypes.bfloat16
    query = np.asarray(query, f32)
    key = np.asarray(key, f32)
    value = np.asarray(value, f32)
    rpb_T = np.ascontiguousarray(
        np.asarray(relative_pos_bias, f32).transpose(2, 0, 1))  # [H, S(j), S(i)]
    # additive mask in [j, i] orientation per batch
    mask_ji = np.asarray(mask)[:, 0].transpose(0, 2, 1)
    madd = np.where(mask_ji == 0, MASK_NEG, f32(0.0)).astype(f32)

    scale = f32(1.0 / np.sqrt(HD))
    Wq_s = (np.asarray(Wq, f32) * scale)
    bq_s = (np.asarray(bq, f32) * scale)
    Wk = np.asarray(Wk, f32)
    Wv = np.asarray(Wv, f32)
    Wo = np.asarray(Wo, f32)
    bk = np.asarray(bk, f32)
    bv = np.asarray(bv, f32)
    ones64 = np.ones((1, 64), mmdt)

    in_maps = []
    for c in range(NCORES):
        b = c // 4
        h0 = (c % 4) * HPC
        cols = slice(h0 * HD, (h0 + HPC) * HD)
        in_maps.append({
            "q_in": query[b].astype(mmdt),
            "k_in": key[b].astype(mmdt),
            "v_in": value[b].astype(mmdt),
            "wq": np.ascontiguousarray(Wq_s[:, cols]).astype(mmdt),
            "wk": np.ascontiguousarray(Wk[:, cols]).astype(mmdt),
            "wv": np.ascontiguousarray(Wv[:, cols]).astype(mmdt),
            "wo": np.ascontiguousarray(Wo[cols, :]).astype(mmdt),
            "bq": np.ascontiguousarray(bq_s[cols]),
            "bk": np.ascontiguousarray(bk[cols]),
            "bv_rep": np.ascontiguousarray(
                np.broadcast_to(bv[cols], (P, HDC))),
            "ones64": ones64,
            "bias_c": np.ascontiguousarray(
                (rpb_T[h0:h0 + HPC] + madd[b][None])
                .reshape(HPC, NJT, P, NISLAB, ISLAB)
                .transpose(0, 1, 3, 2, 4)).astype(biasdt),
        })
    return in_maps


def run_sharded(run_kwargs=None, **inputs):
    """Build + run on 8 cores; returns (output, BassKernelResults)."""
    from concourse.bass_utils import run_bass_kernel_spmd

    nc = _get_nc()
    in_maps = _prep_inputs(**inputs)
    res = run_bass_kernel_spmd(nc, in_maps, list(range(NCORES)),
                               **(run_kwargs or {}))
    bo = np.asarray(inputs["bo"], np.float32)
    out = np.zeros((B, S, D), np.float32)
    for c in range(NCORES):
        out[c // 4] += res.results[c]["out_p"]
    out += bo[None, None, :]
    return out, res


def kernel(**inputs):
    out, _ = run_sharded(**inputs)
    return out


# revision 13
# speedup vs baseline: 1.0227x; 1.0227x over previous
"""Multi-head attention TRN2 kernel, sharded over 8 NeuronCores.

Sharding: (batch, head-group) — core c handles batch c//4 and heads
(c%4)*4 .. (c%4)*4+3. Each core computes its 4 heads' attention plus its
partial output projection; the host sums the 4 partials per batch and adds bo.

Device-side layout (per core):
  QT, KT: [hd=256, S] (bf16)   -- projections computed transposed
  V:      [S, 4 heads, 65]     -- 64 hd cols + ones col (softmax denominator)
  scores computed transposed [j, i] so the softmax sum and attn@V both
  contract over j (the partition dim) with no transposes of the big matrices.
  relative_pos_bias arrives pre-sliced per head in [h, j, i] layout (bf16)
  with the attention mask folded in as an additive -30000 (host-side prep).
  Matmuls run in bf16 (single-pass on the PE); all accumulation is fp32 in
  PSUM. Set PRECISE=1 to switch the matmul path to f32r (2-pass, ~2x slower,
  rel err ~3e-4 instead of ~4e-3).
"""
import os
import sys

if "/opt/trn_rl_repo" not in sys.path:
    sys.path.insert(0, "/opt/trn_rl_repo")

from contextlib import ExitStack

import ml_dtypes
import numpy as np

B, S, D, H = 2, 2048, 1024, 16
HD = D // H          # 64
NCORES = 8
HPC = 4              # heads per core
HDC = HPC * HD       # 256 head-dim cols per core
P = 128
ISLAB = 512          # i-columns per score slab
NJT = S // P         # 16 j tiles
NISLAB = S // ISLAB  # 4 i slabs
NSC = S // ISLAB     # 4 seq chunks in stage 0
MASK_NEG = np.float32(-30000.0)

PRECISE = os.environ.get("PRECISE", "0") == "1"

_CACHE = {}


def _build():
    import concourse.bass as bass
    import concourse.mybir as mybir
    import concourse.tile as tile
    from concourse.tile import add_dep_helper
    from concourse import bacc
    from concourse.masks import make_identity

    f32 = mybir.dt.float32
    mmdt = mybir.dt.float32r if PRECISE else mybir.dt.bfloat16
    biasdt = f32 if PRECISE else mybir.dt.bfloat16

    # All our ACT funcs (Exp, Ln, Copy, Identity) live together in the
    # 'natural_log_exp_and_others' table set; restricting the registry to it
    # makes insert_act_table_loads emit ONE load instead of thrashing
    # exp<->ln tables at every softmax-normalization point.
    import concourse.hw_specs as hw_specs
    if not getattr(hw_specs, "_mha_table_patch", False):
        _orig_gat = hw_specs.get_activation_tables

        def _one_table(arch, _orig=_orig_gat):
            t = _orig(arch)
            name = "natural_log_exp_and_others"
            if name not in t:
                return t
            keep = t[name]
            # preserve dict size/order (set index == act_func_set_id); just
            # make the shared funcs resolvable only via the ln+exp set
            return {
                k: (v if k == name else (v - keep))
                for k, v in t.items()
            }

        hw_specs.get_activation_tables = _one_table
        bacc.get_activation_tables = _one_table
        hw_specs._mha_table_patch = True

    nc = bacc.Bacc(None, target_bir_lowering=False)

    q_in = nc.declare_dram_parameter("q_in", [S, D], mmdt, isOutput=False)
    k_in = nc.declare_dram_parameter("k_in", [S, D], mmdt, isOutput=False)
    v_in = nc.declare_dram_parameter("v_in", [S, D], mmdt, isOutput=False)
    wq = nc.declare_dram_parameter("wq", [D, HDC], mmdt, isOutput=False)
    wk = nc.declare_dram_parameter("wk", [D, HDC], mmdt, isOutput=False)
    wv = nc.declare_dram_parameter("wv", [D, HDC], mmdt, isOutput=False)
    wo = nc.declare_dram_parameter("wo", [HDC, D], mmdt, isOutput=False)
    bq = nc.declare_dram_parameter("bq", [HDC], f32, isOutput=False)
    bk = nc.declare_dram_parameter("bk", [HDC], f32, isOutput=False)
    bv_rep = nc.declare_dram_parameter("bv_rep", [P, HDC], f32, isOutput=False)
    ones64 = nc.declare_dram_parameter("ones64", [1, 64], mmdt, isOutput=False)
    bias_c = nc.declare_dram_parameter(
        "bias_c", [HPC, NJT, NISLAB, P, ISLAB], biasdt, isOutput=False)
    out_p = nc.declare_dram_parameter("out_p", [S, D], f32, isOutput=True)

    EXP = mybir.ActivationFunctionType.Exp
    LN = mybir.ActivationFunctionType.Ln

    with tile.TileContext(nc) as tc, ExitStack() as big:
        consts = big.enter_context(tc.tile_pool(name="consts", bufs=1))
        persist = big.enter_context(tc.tile_pool(name="persist", bufs=1))

        ident = consts.tile([P, P], mmdt)
        make_identity(nc, ident)
        ones64_sb = consts.tile([1, 64], mmdt)
        nc.sync.dma_start(ones64_sb, ones64[:])
        bqv = consts.tile([P, 2], f32)
        nc.sync.dma_start(bqv, bq[:].rearrange("(o p) -> p o", p=P))
        bkv = consts.tile([P, 2], f32)
        nc.sync.dma_start(bkv, bk[:].rearrange("(o p) -> p o", p=P))
        bv_sb = consts.tile([P, HDC], f32)
        nc.sync.dma_start(bv_sb, bv_rep[:])
        ones_col = consts.tile([P, 1], f32)
        nc.vector.memset(ones_col, 1.0)

        wq_sb = consts.tile([P, 8, HDC], mmdt)
        nc.sync.dma_start(wq_sb, wq[:].rearrange("(dk p) m -> p dk m", p=P))
        wk_sb = consts.tile([P, 8, HDC], mmdt)
        nc.sync.dma_start(wk_sb, wk[:].rearrange("(dk p) m -> p dk m", p=P))
        wv_sb = consts.tile([P, 8, HDC], mmdt)
        nc.sync.dma_start(wv_sb, wv[:].rearrange("(dk p) m -> p dk m", p=P))
        wo_sb = consts.tile([P, 2, D], mmdt)
        nc.sync.dma_start(wo_sb, wo[:].rearrange("(kt p) n -> p kt n", p=P))

        qt_full = persist.tile([P, 2, S], mmdt)   # [hd%128, hd//128, seq]
        kt_full = persist.tile([P, 2, S], mmdt)
        v_full = persist.tile([P, NJT, HPC, HD + 1], mmdt)  # [seq%128, jt, h, hd|1]
        ctxT = persist.tile([P, 2, S], mmdt)      # [hd%128, hd//128, i]

        # ones column of V (softmax denominator trick)
        for jt in range(NJT):
            nc.vector.tensor_copy(
                v_full[:, jt, :, HD:HD + 1],
                ones_col[:, None, :].to_broadcast((P, HPC, 1)))

        # ---------------- Stage 0: transpose inputs + projections ----------
        s0 = big.enter_context(tc.tile_pool(name="s0", bufs=6))
        s0x = big.enter_context(tc.tile_pool(name="s0x", bufs=2))
        with ExitStack() as st0:
            tps = st0.enter_context(tc.tile_pool(name="tps", bufs=2, space="PSUM"))
            pps = st0.enter_context(tc.tile_pool(name="pps", bufs=2, space="PSUM"))
            vps = st0.enter_context(tc.tile_pool(name="vps", bufs=2, space="PSUM"))

            for x_dram, which in ((q_in, "q"), (k_in, "k"), (v_in, "v")):
                for sc in range(NSC):
                    xn = []
                    for st in range(4):
                        t = s0.tile([P, D], mmdt, tag="xn")
                        nc.sync.dma_start(t, x_dram[sc * ISLAB + st * P:
                                                    sc * ISLAB + (st + 1) * P, :])
                        xn.append(t)
                    xT = s0x.tile([P, 8, ISLAB], mmdt, tag="xT")
                    for dk in range(8):
                        tp4 = tps.tile([P, ISLAB], mmdt, tag="tp4")
                        for st in range(4):
                            nc.tensor.transpose(
                                tp4[:, st * P:(st + 1) * P],
                                xn[st][:, dk * P:(dk + 1) * P], ident)
                        if dk % 2 == 0:
                            nc.scalar.copy(xT[:, dk, :], tp4)
                        else:
                            nc.vector.tensor_copy(xT[:, dk, :], tp4)
                    if which == "v":
                        for st in range(4):
                            vp = vps.tile([P, HDC], f32, tag="vp")
                            for dk in range(8):
                                nc.tensor.matmul(
                                    vp, xT[:, dk, st * P:(st + 1) * P],
                                    wv_sb[:, dk, :],
                                    start=(dk == 0), stop=(dk == 7),
                                )
                            jt = sc * 4 + st
                            nc.vector.tensor_add(
                                v_full[:, jt, :, :HD],
                                vp.rearrange("p (h d) -> p h d", h=HPC),
                                bv_sb.rearrange("p (h d) -> p h d", h=HPC),
                            )
                    else:
                        dst = qt_full if which == "q" else kt_full
                        w_sb = wq_sb if which == "q" else wk_sb
                        bvec = bqv if which == "q" else bkv
                        for mt in range(2):
                            pp = pps.tile([P, ISLAB], f32, tag="pp")
                            for dk in range(8):
                                nc.tensor.matmul(
                                    pp, w_sb[:, dk, mt * P:(mt + 1) * P],
                                    xT[:, dk, :],
                                    start=(dk == 0), stop=(dk == 7),
                                )
                            nc.vector.tensor_scalar_add(
                                dst[:, mt, sc * ISLAB:(sc + 1) * ISLAB],
                                pp, bvec[:, mt:mt + 1],
                            )

        # ---------------- Stage 1: attention ------------------------------
        sbias = big.enter_context(tc.tile_pool(name="sbias", bufs=8))
        sein = big.enter_context(tc.tile_pool(name="sein", bufs=6))
        sexp = big.enter_context(tc.tile_pool(name="sexp", bufs=8))
        snrm = big.enter_context(tc.tile_pool(name="snrm", bufs=2))
        dnrm = big.enter_context(tc.tile_pool(name="dnrm", bufs=2, space="DRAM"))
        with ExitStack() as st1:
            sps = st1.enter_context(tc.tile_pool(name="sps", bufs=6, space="PSUM"))
            cps = st1.enter_context(tc.tile_pool(name="cps", bufs=2, space="PSUM"))

            LAG = 5

            def make_norm_steps(cp, hp, ho, isl):
                # Deferred softmax normalization for one finished slab:
                # 1/denom via exp(-ln(d)) on ACT, partition-broadcast via a
                # DRAM bounce, multiply on DVE. Emitted as discrete steps that
                # the caller interleaves into the NEXT slab's loop so these
                # never stall the in-order ACT/PE streams at slab boundaries.
                state = {}

                def s_ln():
                    state["lns"] = snrm.tile([1, ISLAB], f32, tag="lns", name="lns")
                    nc.scalar.activation(state["lns"], cp[HD:HD + 1, :], LN)

                def s_recip():
                    state["recip"] = snrm.tile([1, ISLAB], f32, tag="recip", name="recip")
                    nc.scalar.activation(state["recip"], state["lns"], EXP,
                                         scale=-1.0)

                def s_dma1():
                    state["dns"] = dnrm.tile([1, ISLAB], f32, tag="dns", name="dns")
                    nc.sync.dma_start(state["dns"], state["recip"])

                def s_dma2():
                    dns = state["dns"]
                    state["bsb"] = snrm.tile([64, ISLAB], f32, tag="bsb", name="bsb")
                    nc.sync.dma_start(state["bsb"], bass.AP(
                        tensor=dns.tensor, offset=dns.offset,
                        ap=[[0, 64]] + list(dns[0].ap)))

                def s_mul():
                    nc.vector.tensor_mul(
                        ctxT[hp:hp + 64, ho, isl * ISLAB:(isl + 1) * ISLAB],
                        cp[:HD, :], state["bsb"],
                    )

                return [s_ln, s_recip, s_dma1, s_dma2, s_mul]

            pending = []
            NORM_AT = {3: 0, 6: 1, 9: 2, 12: 3, 15: 4}
            for h in range(HPC):
                hp = (h % 2) * 64       # base partition of this head in qt/kt
                ho = h // 2             # outer index
                for isl in range(NISLAB):
                    qt_h = qt_full[hp:hp + 64, ho, isl * ISLAB:(isl + 1) * ISLAB]
                    cp = cps.tile([HD + 1, ISLAB], f32, tag="cp")
                    ets = [None] * NJT
                    sc_insts = [None] * NJT
                    for jt in range(NJT + LAG):
                        if jt in NORM_AT and pending:
                            pending[NORM_AT[jt]]()
                        if jt < NJT:
                            sp = sps.tile([P, ISLAB], f32, tag="sp")
                            smm = nc.tensor.matmul(
                                sp, kt_full[hp:hp + 64, ho, jt * P:(jt + 1) * P],
                                qt_h, start=True, stop=True,
                            )
                            sc_insts[jt] = smm
                            bt = sbias.tile([P, ISLAB], biasdt, tag="bt")
                            nc.sync.dma_start(bt, bias_c[h, jt, isl])
                            ein = sein.tile([P, ISLAB], f32, tag="ein")
                            nc.vector.tensor_add(ein, sp, bt)
                            et = sexp.tile([P, ISLAB], mmdt, tag="et")
                            nc.scalar.activation(et, ein, EXP)
                            ets[jt] = et
                        if jt >= LAG:
                            j2 = jt - LAG
                            cmm = nc.tensor.matmul(
                                cp, v_full[:, j2, h, :], ets[j2],
                                start=(j2 == 0), stop=(j2 == NJT - 1),
                            )
                            if jt < NJT:
                                # keep the software-pipeline skew in the PE
                                # stream: ctx(j2) goes AFTER scores(j2+LAG)
                                add_dep_helper(
                                    sc_insts[jt].ins, cmm.ins, sync=False,
                                    reason="preserve scores/ctx LAG skew")
                    pending = make_norm_steps(cp, hp, ho, isl)
            for step in pending:
                step()

        # ---------------- Stage 2: output projection ----------------------
        so = big.enter_context(tc.tile_pool(name="so", bufs=3))
        with ExitStack() as st2:
            ops = st2.enter_context(tc.tile_pool(name="ops", bufs=2, space="PSUM"))
            for it in range(S // P):
                for nt in range(2):
                    op = ops.tile([P, ISLAB], f32, tag="op")
                    for kt in range(2):
                        nc.tensor.matmul(
                            op, ctxT[:, kt, it * P:(it + 1) * P],
                            wo_sb[:, kt, nt * ISLAB:(nt + 1) * ISLAB],
                            start=(kt == 0), stop=(kt == 1),
                        )
                    ot = so.tile([P, ISLAB], f32, tag="ot")
                    nc.scalar.copy(ot, op)
                    nc.sync.dma_start(
                        out_p[it * P:(it + 1) * P,
                              nt * ISLAB:(nt + 1) * ISLAB], ot)

    nc.compile()
    return nc


def _get_nc():
    if "nc" not in _CACHE:
        _CACHE["nc"] = _build()
    return _CACHE["nc"]


def _prep_inputs(query, key, value, mask, relative_pos_bias,
                 Wq, bq, Wk, bk, Wv, bv, Wo, bo):
    f32 = np.float32
    mmdt = f32 if PRECISE else ml_dtypes.bfloat16
    biasdt = f32 if PRECISE else ml_dtypes.bfloat16
    query = np.asarray(query, f32)
    key = np.asarray(key, f32)
    value = np.asarray(value, f32)
    rpb_T = np.ascontiguousarray(
        np.asarray(relative_pos_bias, f32).transpose(2, 0, 1))  # [H, S(j), S(i)]
    # additive mask in [j, i] orientation per batch
    mask_ji = np.asarray(mask)[:, 0].transpose(0, 2, 1)
    madd = np.where(mask_ji == 0, MASK_NEG, f32(0.0)).astype(f32)

    scale = f32(1.0 / np.sqrt(HD))
    Wq_s = (np.asarray(Wq, f32) * scale)
    bq_s = (np.asarray(bq, f32) * scale)
    Wk = np.asarray(Wk, f32)
    Wv = np.asarray(Wv, f32)
    Wo = np.asarray(Wo, f32)
    bk = np.asarray(bk, f32)
    bv = np.asarray(bv, f32)
    ones64 = np.ones((1, 64), mmdt)

    in_maps = []
    for c in range(NCORES):
        b = c // 4
        h0 = (c % 4) * HPC
        cols = slice(h0 * HD, (h0 + HPC) * HD)
        in_maps.append({
            "q_in": query[b].astype(mmdt),
            "k_in": key[b].astype(mmdt),
            "v_in": value[b].astype(mmdt),
            "wq": np.ascontiguousarray(Wq_s[:, cols]).astype(mmdt),
            "wk": np.ascontiguousarray(Wk[:, cols]).astype(mmdt),
            "wv": np.ascontiguousarray(Wv[:, cols]).astype(mmdt),
            "wo": np.ascontiguousarray(Wo[cols, :]).astype(mmdt),
            "bq": np.ascontiguousarray(bq_s[cols]),
            "bk": np.ascontiguousarray(bk[cols]),
            "bv_rep": np.ascontiguousarray(
                np.broadcast_to(bv[cols], (P, HDC))),
            "ones64": ones64,
            "bias_c": np.ascontiguousarray(
                (rpb_T[h0:h0 + HPC] + madd[b][None])
                .reshape(HPC, NJT, P, NISLAB, ISLAB)
                .transpose(0, 1, 3, 2, 4)).astype(biasdt),
        })
    return in_maps


def run_sharded(run_kwargs=None, **inputs):
    """Build + run on 8 cores; returns (output, BassKernelResults)."""
    from concourse.bass_utils import run_bass_kernel_spmd

    nc = _get_nc()
    in_maps = _prep_inputs(**inputs)
    res = run_bass_kernel_spmd(nc, in_maps, list(range(NCORES)),
                               **(run_kwargs or {}))
    bo = np.asarray(inputs["bo"], np.float32)
    out = np.zeros((B, S, D), np.float32)
    for c in range(NCORES):
        out[c // 4] += res.results[c]["out_p"]
    out += bo[None, None, :]
    return out, res


def kernel(**inputs):
    out, _ = run_sharded(**inputs)
    return out
